# revision 6
# baseline (speedup 1.0000x reference)
"""GAU (gated attention unit) Bass kernel for Trainium2, 8 NeuronCores.

Sharding: 8 cores = 4 batches x 2 sequence halves. Each core computes the
full k/v for its batch and the attention output rows for its half.

Per-core pipeline:
  phase A/B (streamed per 4-seq-tile group): load x, layernorm (bn_stats),
    PE-transpose to get d-on-partitions, project to v (bf16), kT/qT (fp32r)
    and gate (fp32, spilled to HBM).
  phase C (per 512-row i-block): simT = kT.T @ qT in fp32r, A = relu(sim)^2
    stored bf16, V = A.T @ v in bf16, V *= gate, PE-transpose, out = Vg @ Wo
    in fp32.
"""
import sys

sys.path.insert(0, "/opt/trn_rl_repo")

import numpy as np

import concourse.bass as bass
import concourse.mybir as mybir
from concourse import bacc
from concourse.masks import make_identity
from concourse.tile import TileContext

F32 = mybir.dt.float32
F32R = mybir.dt.float32r
BF16 = mybir.dt.bfloat16
AF = mybir.ActivationFunctionType
OP = mybir.AluOpType

S = 4096          # full sequence
SH = 2048         # per-core q rows
D = 512           # model dim
HID = 1024        # v / gate width
H2 = 2048         # 2*HID
QK = 128
OUT = 8
NKV = S // 128    # 32 kv seq tiles
NQ = SH // 128    # 16 q seq tiles
NCORES = 8

_nc_cache = None


def _ln_transpose_tile(nc, pools, x_dram, row0, normT_grp, t, consts):
    """Load one [128, 512] tile, layernorm, transpose into normT_grp[:, :, t*128:...]."""
    identity, eps_t, g_col, b_col = consts
    xp, sp, npool, psTr = pools
    xt = xp.tile([128, D], F32, tag="xt")
    nc.sync.dma_start(out=xt, in_=x_dram[row0 + t * 128 : row0 + (t + 1) * 128, :])
    st = sp.tile([128, 6], F32, tag="bnst")
    nc.vector.bn_stats(out=st, in_=xt)
    mv = sp.tile([128, 2], F32, tag="bnmv")
    nc.vector.bn_aggr(out=mv, in_=st)
    rstd = sp.tile([128, 1], F32, tag="rstd")
    nc.scalar.activation(out=rstd, in_=mv[:, 1:2], func=AF.Sqrt, bias=eps_t)
    rstd2 = sp.tile([128, 1], F32, tag="rstd2")
    nc.vector.reciprocal(out=rstd2, in_=rstd)
    nmr = sp.tile([128, 1], F32, tag="nmr")
    nc.vector.tensor_scalar(nmr, mv[:, 0:1], rstd2, -1.0, OP.mult, OP.mult)
    nsc = npool.tile([128, D], F32, tag="nsc")
    nc.scalar.activation(out=nsc, in_=xt, func=AF.Identity, bias=nmr, scale=rstd2)
    ptr = psTr.tile([128, 4, 128], F32, tag="ptr")
    for c in range(4):
        nc.tensor.transpose(ptr[:, c, :], nsc[:, c * 128 : (c + 1) * 128], identity)
    for c in range(4):
        nc.vector.tensor_scalar(
            normT_grp[:, c, t * 128 : (t + 1) * 128],
            ptr[:, c, :],
            g_col[:, c : c + 1],
            b_col[:, c : c + 1],
            OP.mult,
            OP.add,
        )


def _build():
    nc = bacc.Bacc()

    xkv = nc.dram_tensor("xkv", [S, D], F32, kind="ExternalInput")
    xq = nc.dram_tensor("xq", [SH, D], F32, kind="ExternalInput")
    wh = nc.dram_tensor("wh", [D, H2], F32, kind="ExternalInput")
    bh = nc.dram_tensor("bh", [1, H2], F32, kind="ExternalInput")
    wqk = nc.dram_tensor("wqk", [D, QK], F32, kind="ExternalInput")
    bqk = nc.dram_tensor("bqk", [QK, 1], F32, kind="ExternalInput")
    lng = nc.dram_tensor("lng", [128, 4], F32, kind="ExternalInput")
    lnb = nc.dram_tensor("lnb", [128, 4], F32, kind="ExternalInput")
    gam0 = nc.dram_tensor("gam0", [QK, 1], F32, kind="ExternalInput")
    bet0 = nc.dram_tensor("bet0", [QK, 1], F32, kind="ExternalInput")
    gam1 = nc.dram_tensor("gam1", [QK, 1], F32, kind="ExternalInput")
    bet1 = nc.dram_tensor("bet1", [QK, 1], F32, kind="ExternalInput")
    wo = nc.dram_tensor("wo", [128, 8, OUT], F32, kind="ExternalInput")
    bo = nc.dram_tensor("bo", [1, OUT], F32, kind="ExternalInput")
    out_d = nc.dram_tensor("out", [SH, OUT], F32, kind="ExternalOutput")

    gate_spill = nc.dram_tensor("gate_spill", [SH, HID], F32)

    with TileContext(nc) as tc:
        with (
            tc.tile_pool(name="persist", bufs=1) as pers,
            tc.tile_pool(name="vpool", bufs=1) as vpool,
        ):
            # ---- persistent tiles ----
            identity = pers.tile([128, 128], F32, tag="ident")
            make_identity(nc, identity)
            eps_t = pers.tile([128, 1], F32, tag="eps")
            nc.vector.memset(eps_t, 1e-5)
            ones_f32 = pers.tile([1, 128], F32, tag="ones32")
            nc.vector.memset(ones_f32, 1.0)
            ones_row = pers.tile([1, 128], F32R, tag="ones")
            nc.vector.tensor_copy(out=ones_row, in_=ones_f32)

            g_col = pers.tile([128, 4], F32, tag="gcol")
            nc.sync.dma_start(out=g_col, in_=lng[:])
            b_col = pers.tile([128, 4], F32, tag="bcol")
            nc.sync.dma_start(out=b_col, in_=lnb[:])
            bqk_col = pers.tile([128, 1], F32, tag="bqk")
            nc.sync.dma_start(out=bqk_col, in_=bqk[:])
            gam0_c = pers.tile([128, 1], F32, tag="g0")
            nc.sync.dma_start(out=gam0_c, in_=gam0[:])
            bet0_c = pers.tile([128, 1], F32, tag="be0")
            nc.sync.dma_start(out=bet0_c, in_=bet0[:])
            gam1_c = pers.tile([128, 1], F32, tag="g1")
            nc.sync.dma_start(out=gam1_c, in_=gam1[:])
            bet1_c = pers.tile([128, 1], F32, tag="be1")
            nc.sync.dma_start(out=bet1_c, in_=bet1[:])
            wo_t = pers.tile([128, 8, OUT], F32, tag="wo")
            nc.sync.dma_start(out=wo_t, in_=wo[:])
            bo_bc = pers.tile([128, OUT], F32, tag="bo")
            nc.sync.dma_start(out=bo_bc, in_=bo[:].to_broadcast([128, OUT]))

            # persistent activations
            v_sb = vpool.tile([128, NKV, HID], BF16, tag="v")
            kt_sb = pers.tile([128, S], F32R, tag="kt")
            qt_sb = pers.tile([128, SH], F32R, tag="qt")

            consts = (identity, eps_t, g_col, b_col)

            # ---- phase A/B: projections ----
            with (
                tc.tile_pool(name="xp", bufs=3) as xp,
                tc.tile_pool(name="sp", bufs=4) as sp,
                tc.tile_pool(name="np", bufs=2) as npool,
                tc.tile_pool(name="nT", bufs=2) as nTp,
                tc.tile_pool(name="zp", bufs=2) as zp,
                tc.tile_pool(name="gp", bufs=2) as gp,
                tc.tile_pool(name="wp", bufs=1) as wp,
                tc.tile_pool(name="psTr", bufs=2, space="PSUM") as psTr,
                tc.tile_pool(name="psZ", bufs=2, space="PSUM") as psZ,
                tc.tile_pool(name="psP", bufs=2, space="PSUM") as psP,
            ):
                apools = (xp, sp, npool, psTr)
                # weights staged + rounded to fp32r
                wh_stage = wp.tile([128, H2], F32, tag="whs")
                whr = wp.tile([128, 4, H2], F32R, tag="whr")
                for c in range(4):
                    nc.sync.dma_start(
                        out=wh_stage, in_=wh[c * 128 : (c + 1) * 128, :]
                    )
                    nc.vector.tensor_copy(out=whr[:, c, :], in_=wh_stage)
                wqk_stage = wp.tile([128, QK], F32, tag="wqs")
                wqkr = wp.tile([128, 4, QK], F32R, tag="wqkr")
                for c in range(4):
                    nc.sync.dma_start(
                        out=wqk_stage, in_=wqk[c * 128 : (c + 1) * 128, :]
                    )
                    nc.vector.tensor_copy(out=wqkr[:, c, :], in_=wqk_stage)
                bh_stage = wp.tile([1, H2], F32, tag="bhs")
                nc.sync.dma_start(out=bh_stage, in_=bh[:])
                bh_row = wp.tile([1, H2], F32R, tag="bhr")
                nc.vector.tensor_copy(out=bh_row, in_=bh_stage)

                # kv loop: 8 groups of 4 seq tiles
                for g in range(NKV // 4):
                    nT = nTp.tile([128, 4, 512], F32R, tag="nT")
                    for t in range(4):
                        _ln_transpose_tile(
                            nc, apools, xkv, g * 512, nT, t, consts
                        )
                    # Z -> kT
                    psz = psZ.tile([128, 512], F32, tag="psz")
                    for c in range(4):
                        nc.tensor.matmul(
                            psz, wqkr[:, c, :], nT[:, c, :],
                            start=(c == 0), stop=(c == 3),
                        )
                    zs = zp.tile([128, 512], F32, tag="zs")
                    nc.scalar.activation(out=zs, in_=psz, func=AF.Silu, bias=bqk_col)
                    nc.vector.tensor_scalar(
                        kt_sb[:, g * 512 : (g + 1) * 512], zs,
                        gam1_c, bet1_c, OP.mult, OP.add,
                    )
                    # v projection, natural layout, bf16
                    for t in range(4):
                        s_idx = g * 4 + t
                        psp = psP.tile([128, HID], F32, tag="psp")
                        for nh in range(2):
                            for c in range(4):
                                nc.tensor.matmul(
                                    psp[:, nh * 512 : (nh + 1) * 512],
                                    nT[:, c, t * 128 : (t + 1) * 128],
                                    whr[:, c, nh * 512 : (nh + 1) * 512],
                                    start=(c == 0), stop=False,
                                )
                            nc.tensor.matmul(
                                psp[:, nh * 512 : (nh + 1) * 512],
                                ones_row,
                                bh_row[0:1, nh * 512 : (nh + 1) * 512],
                                start=False, stop=True,
                            )
                        nc.scalar.activation(
                            out=v_sb[:, s_idx, :], in_=psp, func=AF.Silu
                        )

                # q loop: 4 groups of 4 seq tiles
                for g in range(NQ // 4):
                    nT = nTp.tile([128, 4, 512], F32R, tag="nT")
                    for t in range(4):
                        _ln_transpose_tile(nc, apools, xq, g * 512, nT, t, consts)
                    psz = psZ.tile([128, 512], F32, tag="psz")
                    for c in range(4):
                        nc.tensor.matmul(
                            psz, wqkr[:, c, :], nT[:, c, :],
                            start=(c == 0), stop=(c == 3),
                        )
                    zs = zp.tile([128, 512], F32, tag="zs")
                    nc.scalar.activation(out=zs, in_=psz, func=AF.Silu, bias=bqk_col)
                    nc.vector.tensor_scalar(
                        qt_sb[:, g * 512 : (g + 1) * 512], zs,
                        gam0_c, bet0_c, OP.mult, OP.add,
                    )
                    # gate projection -> HBM spill
                    for t in range(4):
                        i_idx = g * 4 + t
                        psp = psP.tile([128, HID], F32, tag="psp")
                        for nh in range(2):
                            for c in range(4):
                                nc.tensor.matmul(
                                    psp[:, nh * 512 : (nh + 1) * 512],
                                    nT[:, c, t * 128 : (t + 1) * 128],
                                    whr[:, c, HID + nh * 512 : HID + (nh + 1) * 512],
                                    start=(c == 0), stop=False,
                                )
                            nc.tensor.matmul(
                                psp[:, nh * 512 : (nh + 1) * 512],
                                ones_row,
                                bh_row[0:1, HID + nh * 512 : HID + (nh + 1) * 512],
                                start=False, stop=True,
                            )
                        gsb = gp.tile([128, HID], F32, tag="gsb")
                        nc.scalar.activation(out=gsb, in_=psp, func=AF.Silu)
                        nc.sync.dma_start(
                            out=gate_spill[i_idx * 128 : (i_idx + 1) * 128, :],
                            in_=gsb,
                        )

            # ---- phase C: attention ----
            with (
                tc.tile_pool(name="atp", bufs=2) as atp,
                tc.tile_pool(name="rtp", bufs=2) as rtp,
                tc.tile_pool(name="gsp", bufs=3) as gsp,
                tc.tile_pool(name="vgp", bufs=2) as vgp,
                tc.tile_pool(name="vgtp", bufs=2) as vgtp,
                tc.tile_pool(name="osp", bufs=2) as osp,
                tc.tile_pool(name="psSim", bufs=1, space="PSUM") as psSim,
                tc.tile_pool(name="psV", bufs=2, space="PSUM") as psV,
                tc.tile_pool(name="psT", bufs=1, space="PSUM") as psT,
                tc.tile_pool(name="psO", bufs=1, space="PSUM") as psO,
            ):
                for ib in range(SH // 512):
                    at = atp.tile([128, NKV, 512], BF16, tag="at")
                    for jt in range(NKV):
                        pss = psSim.tile([128, 512], F32, tag="pss")
                        nc.tensor.matmul(
                            pss,
                            kt_sb[:, jt * 128 : (jt + 1) * 128],
                            qt_sb[:, ib * 512 : (ib + 1) * 512],
                            start=True, stop=True,
                        )
                        rt = rtp.tile([128, 512], F32, tag="rt")
                        nc.scalar.activation(out=rt, in_=pss, func=AF.Relu)
                        nc.vector.tensor_mul(out=at[:, jt, :], in0=rt, in1=rt)
                    for t in range(4):
                        i_idx = ib * 4 + t
                        gate_t = gsp.tile([128, HID], F32, tag="gt")
                        nc.sync.dma_start(
                            out=gate_t,
                            in_=gate_spill[i_idx * 128 : (i_idx + 1) * 128, :],
                        )
                        psv = psV.tile([128, HID], F32, tag="psv")
                        for jt in range(NKV):
                            for nh in range(2):
                                nc.tensor.matmul(
                                    psv[:, nh * 512 : (nh + 1) * 512],
                                    at[:, jt, t * 128 : (t + 1) * 128],
                                    v_sb[:, jt, nh * 512 : (nh + 1) * 512],
                                    start=(jt == 0), stop=(jt == NKV - 1),
                                )
                        vg = vgp.tile([128, HID], F32, tag="vg")
                        nc.vector.tensor_mul(out=vg, in0=psv, in1=gate_t)
                        pst = psT.tile([128, 8, 128], F32, tag="pst")
                        for hc in range(8):
                            nc.tensor.transpose(
                                pst[:, hc, :], vg[:, hc * 128 : (hc + 1) * 128],
                                identity,
                            )
                        vgt = vgtp.tile([128, 8, 128], F32, tag="vgt")
                        nc.vector.tensor_copy(out=vgt, in_=pst)
                        pso = psO.tile([128, OUT], F32, tag="pso")
                        for hc in range(8):
                            nc.tensor.matmul(
                                pso, vgt[:, hc, :], wo_t[:, hc, :],
                                start=(hc == 0), stop=(hc == 7),
                            )
                        osb = osp.tile([128, OUT], F32, tag="osb")
                        nc.vector.tensor_add(out=osb, in0=pso, in1=bo_bc)
                        nc.sync.dma_start(
                            out=out_d[i_idx * 128 : (i_idx + 1) * 128, :],
                            in_=osb,
                        )

    nc.compile()
    return nc


def _get_nc():
    global _nc_cache
    if _nc_cache is None:
        _nc_cache = _build()
    return _nc_cache


def _prep_in_maps(inputs):
    return _prep(**inputs)


def _prep(x, ln_g, ln_b, Wh, bh, Wqk, bqk, gamma, beta, Wo, bo):
    x = np.asarray(x, dtype=np.float32)
    f = lambda a: np.ascontiguousarray(np.asarray(a, dtype=np.float32))
    shared = {
        "wh": f(Wh),
        "bh": f(bh).reshape(1, H2),
        "wqk": f(Wqk),
        "bqk": f(bqk).reshape(QK, 1),
        "lng": f(ln_g).reshape(4, 128).T,
        "lnb": f(ln_b).reshape(4, 128).T,
        "gam0": f(gamma[0] / float(S)).reshape(QK, 1),
        "bet0": f(beta[0] / float(S)).reshape(QK, 1),
        "gam1": f(gamma[1]).reshape(QK, 1),
        "bet1": f(beta[1]).reshape(QK, 1),
        "wo": f(Wo).reshape(8, 128, OUT).transpose(1, 0, 2),
        "bo": f(bo).reshape(1, OUT),
    }
    shared = {k: np.ascontiguousarray(v) for k, v in shared.items()}
    in_maps = []
    for c in range(NCORES):
        b, h = c // 2, c % 2
        m = dict(shared)
        m["xkv"] = np.ascontiguousarray(x[b])
        m["xq"] = np.ascontiguousarray(x[b, h * SH : (h + 1) * SH])
        in_maps.append(m)
    return in_maps


def kernel(x, ln_g, ln_b, Wh, bh, Wqk, bqk, gamma, beta, Wo, bo):
    from concourse.bass_utils import run_bass_kernel_spmd

    nc = _get_nc()
    in_maps = _prep(x, ln_g, ln_b, Wh, bh, Wqk, bqk, gamma, beta, Wo, bo)
    res = run_bass_kernel_spmd(nc, in_maps, core_ids=list(range(NCORES)))
    out = np.empty((4, S, OUT), dtype=np.float32)
    for c in range(NCORES):
        b, h = c // 2, c % 2
        out[b, h * SH : (h + 1) * SH] = res.results[c]["out"]
    return out
